# revision 12
# baseline (speedup 1.0000x reference)
"""Trainium2 Bass kernel for nn_ConvAlignLoss (8-core data parallel).

Self-contained: hardcodes shapes; imports concourse from /opt/trn_rl_repo.

Per core (R=64 rows):
  loss_astf partial: sum((pred-true)^2)
  conv = irfft16384(fft(pred) * conj(fft(egf_pad)))[:14337]  (2-stage matmul FFT)
  cc   = irfft32768(fft(conv_pad) * conj(fft(target_pad)))
  shift = mapped masked argmax of cc (== reference argmax over n=28673)
  loss_conv partial: sum((conv[(7040+i+shift) % 14337] - target[7040+i])^2)
Host combines the 8 cores' (sum_astf, sum_conv) into the scalar losses.

FFT structure (N = 128*N2):
  FWD:  D[t1,t2]=x[N2*t1+t2]; A[t2,f1]=sum_t1 D*W1 (data-stationary matmul);
        B=A*tw; Z[f2,f1]=sum_t2 W2[t2,f2]*B.   Z2d[f2,f1] == X[f1+128*f2]
  INV:  G[f1,t2]=sum_f2 S2d[f2,f1]*V2[f2,t2] (S stationary); H=G*itw;
        x2d[t1,t2]=(1/N) Re(sum_f1 V1[f1,t1]*H[f1,t2])

bf16 matmuls with [re|im]-concat const rhs (data stationary); paired
[128,1024] PSUM tiles; Scalar scatter-copies PSUM->bf16 staging
([re...|im...]); Vector does few wide flat bf16 ops (DVE 2x);
block-batched multi-row DMAs; astf on GpSimd.
"""
import sys

sys.path.insert(0, "/opt/trn_rl_repo")

import numpy as np
import concourse.bass as bass
import concourse.bacc as bacc
import concourse.mybir as mybir
from concourse import tile

F32 = mybir.dt.float32
BF16 = mybir.dt.bfloat16
I32 = mybir.dt.int32
AT = mybir.AluOpType
AX = mybir.AxisListType

R = 64
NCORES = 8
L1, L2 = 16384, 2048
CONV_LEN = L1 - L2 + 1      # 14337
N_A, N_B = 16384, 32768
GAP_LO, GAP_HI = CONV_LEN, N_B - CONV_LEN + 1   # gap [14337, 18432)
CROP = 256
START0 = (CONV_LEN - CROP) // 2                 # 7040
PITCH = 14720
BIGL = float(2 ** 23)


def _dft(n, sign):
    k = np.arange(n)
    return np.exp(sign * 2j * np.pi * np.outer(k, k) / n)


def make_consts():
    c = {}

    def put(name, arr, dt=np.float32):
        c[name] = np.ascontiguousarray(np.asarray(arr, np.float64)).astype(dt)

    W1 = _dft(128, -1)
    put("W1r", W1.real); put("W1i", W1.imag); put("nW1i", -W1.imag)
    put("W1ri", np.concatenate([W1.real, W1.imag], axis=1))      # [128,256]
    twA = np.exp(-2j * np.pi * np.outer(np.arange(128), np.arange(128)) / N_A)
    put("twAr", twA.real); put("twAi", twA.imag)
    V2A = _dft(128, +1)
    put("V2ri", np.concatenate([V2A.real, V2A.imag], axis=1))    # [128,256]
    put("V2nir", np.concatenate([-V2A.imag, V2A.real], axis=1))  # [128,256]
    itwA = np.exp(2j * np.pi * np.outer(np.arange(128), np.arange(128)) / N_A)
    put("itwAr", itwA.real); put("itwAi", itwA.imag)
    V1A = _dft(128, +1) / N_A
    put("V1Ar", V1A.real); put("nV1Ai", -V1A.imag)

    W2B = _dft(256, -1)          # [t2, f2]
    for a in range(2):
        for b in range(2):
            blk = W2B[a * 128:(a + 1) * 128, b * 128:(b + 1) * 128]
            put(f"W2Br{a}{b}", blk.real)
            put(f"W2Bi{a}{b}", blk.imag)
            put(f"nW2Bi{a}{b}", -blk.imag)
    twB = np.exp(-2j * np.pi * np.outer(np.arange(256), np.arange(128)) / N_B)
    twB01r = np.concatenate([twB.real[0:128], twB.real[128:256]], axis=1)
    twB01i = np.concatenate([twB.imag[0:128], twB.imag[128:256]], axis=1)
    put("twBr4", np.tile(twB01r, (1, 4)))                        # [128,1024]
    put("twBi4", np.tile(twB01i, (1, 4)))
    V2B = _dft(256, +1)          # [f2, t2]
    for a in range(2):
        blk = V2B[a * 128:(a + 1) * 128, :]
        put(f"V2Bri{a}", np.concatenate([blk.real, blk.imag], axis=1))   # [128,512]
        put(f"V2Bnir{a}", np.concatenate([-blk.imag, blk.real], axis=1))
    itwB = np.exp(2j * np.pi * np.outer(np.arange(128), np.arange(256)) / N_B)
    put("itwBr4", np.tile(itwB.real, (1, 4)))                    # [128,1024]
    put("itwBi4", np.tile(itwB.imag, (1, 4)))
    V1B = _dft(128, +1) / N_B
    put("V1Br", V1B.real); put("nV1Bi", -V1B.imag)

    put("ident", np.eye(128))
    put("ones1x128", np.ones((1, 128)))
    put("ones128", np.ones((128, 1)))
    put("ones64", np.ones((64, 1)))

    j = np.arange(128)[:, None] * 256 + np.arange(256)[None, :]   # [t1, t2]
    gap = (j >= GAP_LO) & (j < GAP_HI)
    put("maskB", np.where(gap, -1e30, 0.0))
    shiftval = np.where(j <= CONV_LEN - 1, j - (CONV_LEN - 1), j - GAP_HI + 1)
    put("shvB", np.where(gap, 0.0, shiftval - BIGL))
    put("winidx", np.arange(R)[:, None] * PITCH
        + np.arange(CROP)[None, :])                               # [64, 256]
    return c


F32_CONST = {"ident", "ones1x128", "ones128", "ones64", "maskB", "shvB",
             "winidx"}


def make_blobs(consts):
    """Pack consts into one bf16 and one f32 [128, W] blob (partition-pad)."""
    import ml_dtypes
    names16 = sorted(n for n in consts if n not in F32_CONST)
    names32 = sorted(n for n in consts if n in F32_CONST)
    out = {}
    for key, names, dt in (("cblob16", names16, ml_dtypes.bfloat16),
                           ("cblob32", names32, np.float32)):
        offs, W = {}, 0
        for n in names:
            offs[n] = W
            W += consts[n].shape[1]
        arr = np.zeros((128, W), dt)
        for n in names:
            a = consts[n]
            arr[:a.shape[0], offs[n]:offs[n] + a.shape[1]] = a.astype(dt)
        out[key] = (arr, offs, W)
    return out


def _b3(ap, n, inner):
    """[128, inner] const AP -> [128, n, inner] broadcast over middle dim."""
    return ap.rearrange("p (a b) -> p a b", a=1).to_broadcast([128, n, inner])


def _scatter_copy(nc, psum, stg, nb, boff, c, b):
    """psum [128, c*2*b] = c chunks of [re(b)|im(b)] -> staging tile
    stg [128, 2*nb*b] laid out [re(nb*b) | im(nb*b)], chunks at boff..boff+c."""
    inv = psum.rearrange("p (c ri b) -> p ri c b", ri=2, b=b)
    outv = stg[:].rearrange("p (ri n b) -> p ri n b", ri=2, b=b)[
        :, :, boff:boff + c, :]
    nc.scalar.copy(outv, inv)


def _cmul_flat(nc, pool, stg, outr, outi, twr, twi, w, tag):
    """stg [128, 2w] = [re(w)|im(w)] bf16; out = (re+i*im)*(twr+i*twi).
    outr/outi [128, w] flat SBUF bf16; twr/twi const APs shaped like [128, w]."""
    sr = stg[:, 0:w]
    si = stg[:, w:2 * w]
    tmp = pool.tile([128, w], BF16, tag=tag, name=tag)
    srv = sr.rearrange("p (a b) -> p a b", b=128)
    siv = si.rearrange("p (a b) -> p a b", b=128)
    tv = tmp[:].rearrange("p (a b) -> p a b", b=128)
    orv = outr.rearrange("p (a b) -> p a b", b=128)
    oiv = outi.rearrange("p (a b) -> p a b", b=128)
    nc.vector.tensor_tensor(orv, srv, twr, op=AT.mult)
    nc.vector.tensor_tensor(tv, siv, twi, op=AT.mult)
    nc.vector.tensor_tensor(orv, orv, tv, op=AT.subtract)
    nc.vector.tensor_tensor(oiv, srv, twi, op=AT.mult)
    nc.vector.tensor_tensor(tv, siv, twr, op=AT.mult)
    nc.vector.tensor_tensor(oiv, oiv, tv, op=AT.add)


def _cprod(nc, eng, outr, outi, tmpb, ar, ai, br, bi, tmp2=None):
    """out = a * conj(b) elementwise; all SBUF APs, same shape.
    Real chain on `eng`; imag chain on GpSimd when tmp2 given (parallel)."""
    e2 = nc.gpsimd if tmp2 is not None else eng
    t2 = tmp2 if tmp2 is not None else tmpb
    eng.tensor_tensor(outr, ar, br, op=AT.mult)
    eng.tensor_tensor(tmpb, ai, bi, op=AT.mult)
    eng.tensor_tensor(outr, outr, tmpb, op=AT.add)
    e2.tensor_tensor(outi, ai, br, op=AT.mult)
    e2.tensor_tensor(t2, ar, bi, op=AT.mult)
    e2.tensor_tensor(outi, outi, t2, op=AT.subtract)


def build_nc(cdt=BF16, rows=R, rbb=8, rb2=4):
    nc = bacc.Bacc("TRN2", target_bir_lowering=False, debug=False,
                   num_devices=NCORES)
    consts = make_consts()

    pred = nc.dram_tensor("pred", [rows, L1], F32, kind="ExternalInput")
    true_ = nc.dram_tensor("true", [rows, L1], F32, kind="ExternalInput")
    egf = nc.dram_tensor("egf", [rows, L2], F32, kind="ExternalInput")
    target = nc.dram_tensor("target", [rows, CONV_LEN], F32,
                            kind="ExternalInput")
    out = nc.dram_tensor("out", [1, 2], F32, kind="ExternalOutput")
    scratch = nc.dram_tensor("scratch", [rows, PITCH], F32)

    blobs = make_blobs(consts)
    cdram = {
        "cblob16": nc.dram_tensor("cblob16", [128, blobs["cblob16"][2]], cdt,
                                  kind="ExternalInput"),
        "cblob32": nc.dram_tensor("cblob32", [128, blobs["cblob32"][2]], F32,
                                  kind="ExternalInput"),
    }

    nb1, nb2 = rows // rbb, rows // rb2

    with tile.TileContext(nc) as tc:
        with (
            tc.tile_pool(name="consts", bufs=1) as cpool,
            tc.tile_pool(name="keep", bufs=1) as kpool,
            tc.tile_pool(name="ps", bufs=4, space="PSUM") as pp,
        ):
            cs = {}
            for key, dt in (("cblob16", cdt), ("cblob32", F32)):
                arr, offs, W = blobs[key]
                t = cpool.tile([128, W], dt, tag=key, name=key)
                nc.sync.dma_start(t[:], cdram[key][:])
                for n, off in offs.items():
                    p = consts[n].shape[0]
                    cs[n] = t[0:p, off:off + consts[n].shape[1]]

            allmax = kpool.tile([128, rows], F32, tag="allmax", name="allmax")
            allmin = kpool.tile([128, rows], F32, tag="allmin", name="allmin")
            ccm_all = kpool.tile([128, rows * 256], BF16, tag="ccm", name="ccm")
            astf_acc = kpool.tile([128, 8], F32, tag="astfacc", name="astfacc")
            shifts = kpool.tile([rows, 1], F32, tag="shifts", name="shifts")
            outt = kpool.tile([1, 2], F32, tag="outt", name="outt")

            # ---------------- A) astf (GpSimd; overlaps everything) -------
            predf = pred.ap().rearrange("r l -> (r l)").rearrange(
                "(p f) -> p f", p=128)
            truef = true_.ap().rearrange("r l -> (r l)").rearrange(
                "(p f) -> p f", p=128)
            fch = rows * L1 // 128 // 8
            with tc.tile_pool(name="astf", bufs=2) as apool:
                for i in range(8):
                    tp = apool.tile([128, fch], F32, tag="ap", name="ap")
                    tt = apool.tile([128, fch], F32, tag="at", name="at")
                    sl = bass.ts(i, fch)
                    nc.sync.dma_start(tp[:], predf[:, sl])
                    nc.sync.dma_start(tt[:], truef[:, sl])
                    nc.gpsimd.tensor_tensor(tt[:], tp[:], tt[:], op=AT.subtract)
                    nc.vector.scalar_tensor_tensor(
                        tp[:], tt[:], 1.0, tt[:], op0=AT.bypass, op1=AT.mult,
                        accum_out=astf_acc[:, i:i + 1])

            # ---------------- B) 16K level ----------------
            with tc.tile_pool(name="p16", bufs=2) as dp:
                for b in range(nb1):
                    r0 = b * rbb
                    Dp = dp.tile([128, rbb * 128], cdt, tag="Dp", name="Dp")
                    De = dp.tile([128, rbb * 128], cdt, tag="De", name="De")
                    nc.scalar.memzero(De[:])
                    Dst = dp.tile([128, rbb * 128], F32, tag="Dst", name="Dst")
                    Est = dp.tile([16, rbb * 128], F32, tag="Est", name="Est")
                    nc.sync.dma_start(
                        Dst[:].rearrange("p (r b) -> p r b", b=128),
                        pred[r0:r0 + rbb, :].rearrange(
                            "r (a b) -> a r b", a=128))
                    nc.sync.dma_start(
                        Est[:].rearrange("p (r b) -> p r b", b=128),
                        egf[r0:r0 + rbb, :].rearrange(
                            "r (a b) -> a r b", a=16))
                    nc.scalar.copy(Dp[:], Dst[:])
                    nc.scalar.copy(De[:16, :], Est[:16, :])

                    # stage 1 + twiddle: per input, 2 psum tiles of 4 chunks
                    Bs = {}
                    twr3 = _b3(cs["twAr"], rbb, 128)
                    twi3 = _b3(cs["twAi"], rbb, 128)
                    for inp, D in (("p", Dp), ("e", De)):
                        stg = dp.tile([128, 2 * rbb * 128], BF16,
                                      tag=f"Astg{inp}", name=f"Astg{inp}")
                        for half in range(2):
                            pa = pp.tile([128, 1024], F32, tag="big", name="big")
                            for j in range(4):
                                qq = half * 4 + j
                                nc.tensor.matmul(
                                    pa[:, bass.ts(j, 256)],
                                    lhsT=D[:, bass.ts(qq, 128)],
                                    rhs=cs["W1ri"], start=True, stop=True)
                            _scatter_copy(nc, pa[:], stg, rbb, half * 4, 4, 128)
                        br = dp.tile([128, rbb * 128], cdt, tag=f"B{inp}r",
                                     name=f"B{inp}r")
                        bi = dp.tile([128, rbb * 128], cdt, tag=f"B{inp}i",
                                     name=f"B{inp}i")
                        _cmul_flat(nc, dp, stg, br[:], bi[:], twr3, twi3,
                                   rbb * 128, "cm16")
                        Bs[inp] = (br, bi)

                    # stage 2: Z[inp] single tile [zr(1024)|zi(1024)]
                    Zs = {}
                    for inp in ("p", "e"):
                        br, bi = Bs[inp]
                        zt = dp.tile([128, 2 * rbb * 128], cdt, tag=f"Z{inp}",
                                     name=f"Z{inp}")
                        for g in range(rbb // 4):
                            gsl = bass.ts(g, 512)
                            pz = pp.tile([128, 1024], F32, tag="big", name="big")
                            nc.tensor.matmul(pz[:, 0:512], lhsT=cs["W1r"],
                                             rhs=br[:, gsl], start=True, stop=False)
                            nc.tensor.matmul(pz[:, 0:512], lhsT=cs["nW1i"],
                                             rhs=bi[:, gsl], start=False, stop=True)
                            nc.tensor.matmul(pz[:, 512:1024], lhsT=cs["W1i"],
                                             rhs=br[:, gsl], start=True, stop=False)
                            nc.tensor.matmul(pz[:, 512:1024], lhsT=cs["W1r"],
                                             rhs=bi[:, gsl], start=False, stop=True)
                            _scatter_copy(nc, pz[:], zt, rbb // 4, g, 1, 512)
                        Zs[inp] = zt

                    # S = Zp * conj(Ze)  (vector, flat bf16)
                    Sr = dp.tile([128, rbb * 128], cdt, tag="Sr", name="Sr")
                    Si = dp.tile([128, rbb * 128], cdt, tag="Si", name="Si")
                    tmpb = dp.tile([128, rbb * 128], BF16, tag="tmpbig",
                                   name="tmpbig")
                    tmpb2 = dp.tile([128, rbb * 128], BF16, tag="tmpbig2",
                                    name="tmpbig2")
                    zp, ze = Zs["p"], Zs["e"]
                    W = rbb * 128
                    _cprod(nc, nc.vector, Sr[:], Si[:], tmpb[:],
                           zp[:, 0:W], zp[:, W:2 * W], ze[:, 0:W], ze[:, W:2 * W],
                           tmp2=tmpb2[:])

                    # inverse G + itw
                    Gstg = dp.tile([128, 2 * rbb * 128], BF16, tag="Gstg",
                                   name="Gstg")
                    for half in range(2):
                        pg = pp.tile([128, 1024], F32, tag="big", name="big")
                        for j in range(4):
                            qq = half * 4 + j
                            sl = bass.ts(qq, 128)
                            nc.tensor.matmul(pg[:, bass.ts(j, 256)],
                                             lhsT=Sr[:, sl], rhs=cs["V2ri"],
                                             start=True, stop=False)
                            nc.tensor.matmul(pg[:, bass.ts(j, 256)],
                                             lhsT=Si[:, sl], rhs=cs["V2nir"],
                                             start=False, stop=True)
                        _scatter_copy(nc, pg[:], Gstg, rbb, half * 4, 4, 128)
                    Hr = dp.tile([128, rbb * 128], cdt, tag="Hr", name="Hr")
                    Hi = dp.tile([128, rbb * 128], cdt, tag="Hi", name="Hi")
                    itwr3 = _b3(cs["itwAr"], rbb, 128)
                    itwi3 = _b3(cs["itwAi"], rbb, 128)
                    _cmul_flat(nc, dp, Gstg, Hr[:], Hi[:], itwr3, itwi3,
                               rbb * 128, "cm16")

                    convSB = dp.tile([128, rbb * 128], F32, tag="convSB",
                                     name="convSB")
                    pc = pp.tile([128, 1024], F32, tag="big", name="big")
                    for g in range(rbb // 4):
                        gsl = bass.ts(g, 512)
                        nc.tensor.matmul(pc[:, gsl], lhsT=cs["V1Ar"],
                                         rhs=Hr[:, gsl], start=True, stop=False)
                        nc.tensor.matmul(pc[:, gsl], lhsT=cs["nV1Ai"],
                                         rhs=Hi[:, gsl], start=False, stop=True)
                    nc.scalar.copy(convSB[:], pc[:])

                    nc.sync.dma_start(
                        scratch[r0:r0 + rbb, 0:14336].rearrange(
                            "r (a b) -> a r b", a=112),
                        convSB[0:112, :].rearrange("p (r b) -> p r b", b=128))
                    nc.sync.dma_start(
                        scratch[r0:r0 + rbb, 14336:14337].rearrange(
                            "r x -> x r"),
                        convSB[112:113, :].rearrange(
                            "p (r b) -> p r b", b=128)[:, :, 0:1])
                    nc.sync.dma_start(
                        scratch[r0:r0 + rbb, 14337:14593].rearrange(
                            "r (a b) -> a r b", a=2),
                        convSB[0:2, :].rearrange("p (r b) -> p r b", b=128))

            # ---------------- C) 32K level ----------------
            with tc.tile_pool(name="p32", bufs=2) as dp:
                for b in range(nb2):
                    r0 = b * rb2
                    D2c = dp.tile([128, rb2 * 256], cdt, tag="D2c", name="D2c")
                    D2t = dp.tile([128, rb2 * 256], cdt, tag="D2t", name="D2t")
                    nc.scalar.memzero(D2c[:])
                    nc.scalar.memzero(D2t[:])
                    tgc = dp.tile([128, rb2 * 256], F32, tag="D2cs", name="D2cs")
                    tgt_ = dp.tile([128, rb2 * 256], F32, tag="D2ts", name="D2ts")
                    nc.scalar.memzero(tgc[:])
                    nc.scalar.memzero(tgt_[:])
                    nc.sync.dma_start(
                        tgc[0:56, :].rearrange("p (r b) -> p r b", b=256),
                        scratch[r0:r0 + rb2, 0:14336].rearrange(
                            "r (a b) -> a r b", a=56))
                    nc.sync.dma_start(
                        tgc[56:57, :].rearrange("p (r b) -> p r b", b=256)[:, :, 0:1],
                        scratch[r0:r0 + rb2, 14336:14337].rearrange("r x -> x r"))
                    nc.sync.dma_start(
                        tgt_[0:56, :].rearrange("p (r b) -> p r b", b=256),
                        target[r0:r0 + rb2, 0:14336].rearrange(
                            "r (a b) -> a r b", a=56))
                    nc.sync.dma_start(
                        tgt_[56:57, :].rearrange("p (r b) -> p r b", b=256)[:, :, 0:1],
                        target[r0:r0 + rb2, 14336:14337].rearrange("r x -> x r"))
                    nc.scalar.copy(D2c[0:57, :], tgc[0:57, :])
                    nc.scalar.copy(D2t[0:57, :], tgt_[0:57, :])

                    # stage 1 + twiddle; chunk (q,c) order; B2 [128, rb2*256]
                    B2 = {}
                    twr3 = cs["twBr4"].rearrange("p (a b) -> p a b", b=128)
                    twi3 = cs["twBi4"].rearrange("p (a b) -> p a b", b=128)
                    for inp, D in (("c", D2c), ("t", D2t)):
                        stg = dp.tile([128, 2 * rb2 * 256], BF16,
                                      tag=f"A2stg{inp}", name=f"A2stg{inp}")
                        for half in range(2):
                            pa = pp.tile([128, 1024], F32, tag="big", name="big")
                            for j in range(4):
                                ch = half * 4 + j
                                dsl = slice(ch * 128, ch * 128 + 128)
                                nc.tensor.matmul(
                                    pa[:, bass.ts(j, 256)],
                                    lhsT=D[:, dsl], rhs=cs["W1ri"],
                                    start=True, stop=True)
                            _scatter_copy(nc, pa[:], stg, rb2 * 2, half * 4, 4, 128)
                        br = dp.tile([128, rb2 * 256], cdt, tag=f"B2r{inp}",
                                     name=f"B2r{inp}")
                        bi = dp.tile([128, rb2 * 256], cdt, tag=f"B2i{inp}",
                                     name=f"B2i{inp}")
                        _cmul_flat(nc, dp, stg, br[:], bi[:], twr3, twi3,
                                   rb2 * 256, "cm32")
                        B2[inp] = (br, bi)

                    # stage 2: Z2[(inp,f2c)] single tiles [zr(512)|zi(512)]
                    Z2 = {}
                    for inp in ("c", "t"):
                        br, bi = B2[inp]
                        for f2c in range(2):
                            zt = dp.tile([128, 2 * rb2 * 128], cdt,
                                         tag=f"Z2{inp}{f2c}", name=f"Z2{inp}{f2c}")
                            pz = pp.tile([128, 1024], F32, tag="big", name="big")
                            for t2c in range(2):
                                brs = br[:].rearrange(
                                    "p (q two f) -> p q two f", two=2, f=128
                                )[:, :, t2c, :]
                                bis = bi[:].rearrange(
                                    "p (q two f) -> p q two f", two=2, f=128
                                )[:, :, t2c, :]
                                nc.tensor.matmul(pz[:, 0:512],
                                                 lhsT=cs[f"W2Br{t2c}{f2c}"],
                                                 rhs=brs, start=(t2c == 0), stop=False)
                                nc.tensor.matmul(pz[:, 0:512],
                                                 lhsT=cs[f"nW2Bi{t2c}{f2c}"],
                                                 rhs=bis, start=False, stop=(t2c == 1))
                                nc.tensor.matmul(pz[:, 512:1024],
                                                 lhsT=cs[f"W2Bi{t2c}{f2c}"],
                                                 rhs=brs, start=(t2c == 0), stop=False)
                                nc.tensor.matmul(pz[:, 512:1024],
                                                 lhsT=cs[f"W2Br{t2c}{f2c}"],
                                                 rhs=bis, start=False, stop=(t2c == 1))
                            nc.scalar.copy(zt[:], pz[:])
                            Z2[(inp, f2c)] = zt

                    # S2 = Zc * conj(Zt) per f2c  (vector, flat bf16)
                    S2 = {}
                    W2_ = rb2 * 128
                    tmpc = dp.tile([128, W2_], BF16, tag="tmpc", name="tmpc")
                    tmpc2 = dp.tile([128, W2_], BF16, tag="tmpc2", name="tmpc2")
                    for f2c in range(2):
                        zc = Z2[("c", f2c)]
                        zt_ = Z2[("t", f2c)]
                        sr = dp.tile([128, W2_], cdt, tag=f"S2r{f2c}",
                                     name=f"S2r{f2c}")
                        si = dp.tile([128, W2_], cdt, tag=f"S2i{f2c}",
                                     name=f"S2i{f2c}")
                        _cprod(nc, nc.vector, sr[:], si[:], tmpc[:],
                               zc[:, 0:W2_], zc[:, W2_:2 * W2_],
                               zt_[:, 0:W2_], zt_[:, W2_:2 * W2_],
                               tmp2=tmpc2[:])
                        S2[f2c] = (sr, si)

                    # inverse: G chunks qq -> [gr(256)|gi(256)]; pairs in psum
                    G2stg = dp.tile([128, 2 * rb2 * 256], BF16, tag="G2stg",
                                    name="G2stg")
                    for half in range(2):
                        pg = pp.tile([128, 1024], F32, tag="big", name="big")
                        for j in range(2):
                            qq = half * 2 + j
                            sl = bass.ts(qq, 128)
                            for f2c in range(2):
                                sr, si = S2[f2c]
                                nc.tensor.matmul(pg[:, bass.ts(j, 512)],
                                                 lhsT=sr[:, sl],
                                                 rhs=cs[f"V2Bri{f2c}"],
                                                 start=(f2c == 0), stop=False)
                                nc.tensor.matmul(pg[:, bass.ts(j, 512)],
                                                 lhsT=si[:, sl],
                                                 rhs=cs[f"V2Bnir{f2c}"],
                                                 start=False, stop=(f2c == 1))
                        _scatter_copy(nc, pg[:], G2stg, rb2, half * 2, 2, 256)
                    H2r = dp.tile([128, rb2 * 256], cdt, tag="H2r", name="H2r")
                    H2i = dp.tile([128, rb2 * 256], cdt, tag="H2i", name="H2i")
                    itwr3 = cs["itwBr4"].rearrange("p (a b) -> p a b", b=128)
                    itwi3 = cs["itwBi4"].rearrange("p (a b) -> p a b", b=128)
                    _cmul_flat(nc, dp, G2stg, H2r[:], H2i[:], itwr3, itwi3,
                               rb2 * 256, "cm32")

                    pcc = pp.tile([128, 1024], F32, tag="big", name="big")
                    for g in range(rb2 // 2):
                        gsl = bass.ts(g, 512)
                        nc.tensor.matmul(pcc[:, gsl], lhsT=cs["V1Br"],
                                         rhs=H2r[:, gsl], start=True, stop=False)
                        nc.tensor.matmul(pcc[:, gsl], lhsT=cs["nV1Bi"],
                                         rhs=H2i[:, gsl], start=False, stop=True)
                    csl = slice(r0 * 256, (r0 + rb2) * 256)
                    ccv = ccm_all[:, csl].rearrange("p (a b) -> p a b", b=256)
                    nc.vector.scalar_tensor_tensor(
                        ccv, pcc[:].rearrange("p (a b) -> p a b", b=256),
                        1.0, _b3(cs["maskB"], rb2, 256),
                        op0=AT.bypass, op1=AT.add)
                    nc.vector.tensor_reduce(
                        allmax[:, r0:r0 + rb2], ccv, axis=AX.X, op=AT.max)

            # ---------------- D) argmax -> shifts ----------------
            with tc.tile_pool(name="amax", bufs=1) as dp:
                pt = pp.tile([rows, 128], F32, tag="big", name="big")
                nc.tensor.transpose(pt[:], allmax[:, 0:rows], cs["ident"])
                tmax = dp.tile([rows, 128], F32, tag="tmax", name="tmax")
                nc.scalar.copy(tmax[:], pt[:])
                rowmax = dp.tile([rows, 1], F32, tag="rowmax", name="rowmax")
                nc.vector.tensor_reduce(rowmax[:], tmax[:], axis=AX.X, op=AT.max)
                prm = pp.tile([1, rows], F32, tag="big", name="big")
                nc.tensor.transpose(prm[:], rowmax[:], cs["ident"][0:rows, 0:rows])
                rmT = dp.tile([1, rows], F32, tag="rmT", name="rmT")
                nc.scalar.copy(rmT[:], prm[:])
                pmb = pp.tile([128, rows], F32, tag="big", name="big")
                nc.tensor.matmul(pmb[:], lhsT=cs["ones1x128"], rhs=rmT[:],
                                 start=True, stop=True)
                Mb = dp.tile([128, rows], F32, tag="Mb", name="Mb")
                nc.scalar.copy(Mb[:], pmb[:])

                eqm = None
                selm = None
                for bb in range(rows // 8):
                    eqm = dp.tile([128, 8 * 256], BF16, tag="eqm", name="eqm",
                                  bufs=2)
                    selm = dp.tile([128, 8 * 256], F32, tag="selm", name="selm",
                                   bufs=2)
                    csl = bass.ts(bb, 8 * 256)
                    mbb = Mb[:, bb * 8:(bb + 1) * 8]\
                        .rearrange("p (a b) -> p a b", b=1)\
                        .to_broadcast([128, 8, 256])
                    ccv = ccm_all[:, csl].rearrange("p (a b) -> p a b", b=256)
                    eng = nc.vector
                    eng.tensor_tensor(
                        eqm[:].rearrange("p (a b) -> p a b", b=256),
                        ccv, mbb, op=AT.is_equal)
                    eng.tensor_tensor(
                        selm[:].rearrange("p (a b) -> p a b", b=256),
                        eqm[:].rearrange("p (a b) -> p a b", b=256),
                        _b3(cs["shvB"], 8, 256), op=AT.mult)
                    nc.vector.tensor_reduce(
                        allmin[:, bb * 8:(bb + 1) * 8],
                        selm[:].rearrange("p (a b) -> p a b", b=256),
                        axis=AX.X, op=AT.min)
                pt2 = pp.tile([rows, 128], F32, tag="big", name="big")
                nc.tensor.transpose(pt2[:], allmin[:, 0:rows], cs["ident"])
                tmin = dp.tile([rows, 128], F32, tag="tmin", name="tmin")
                nc.scalar.copy(tmin[:], pt2[:])
                nc.vector.tensor_reduce(shifts[:], tmin[:], axis=AX.X, op=AT.min)
                nc.vector.tensor_scalar_add(shifts[:], shifts[:],
                                            BIGL + float(START0))

                # start = (7040 + shift) mod 14337
                m1 = dp.tile([rows, 1], F32, tag="m1", name="m1")
                nc.vector.tensor_scalar(out=m1[:], in0=shifts[:], scalar1=0.0,
                                        scalar2=None, op0=AT.is_lt)
                nc.vector.scalar_tensor_tensor(
                    shifts[:], m1[:], float(CONV_LEN), shifts[:],
                    op0=AT.mult, op1=AT.add)
                nc.vector.tensor_scalar(out=m1[:], in0=shifts[:],
                                        scalar1=float(CONV_LEN), scalar2=None,
                                        op0=AT.is_ge)
                nc.vector.scalar_tensor_tensor(
                    shifts[:], m1[:], float(-CONV_LEN), shifts[:],
                    op0=AT.mult, op1=AT.add)

                idxf = dp.tile([rows, CROP], F32, tag="idxf", name="idxf")
                nc.vector.tensor_tensor(idxf[:], cs["winidx"][0:rows, :],
                                        shifts[:].to_broadcast([rows, CROP]),
                                        op=AT.add)
                idxi = dp.tile([rows, CROP], I32, tag="idxi", name="idxi")
                nc.vector.tensor_copy(idxi[:], idxf[:])
                w = dp.tile([rows, CROP], F32, tag="wg", name="wg")
                nc.gpsimd.indirect_dma_start(
                    out=w[:], out_offset=None,
                    in_=scratch.ap().rearrange("r p -> (r p)").rearrange(
                        "(a b) -> a b", b=1),
                    in_offset=bass.IndirectOffsetOnAxis(ap=idxi[:], axis=0),
                )
                tw_ = dp.tile([rows, CROP], F32, tag="twin", name="twin")
                nc.sync.dma_start(tw_[:], target[:, START0:START0 + CROP])
                nc.vector.tensor_tensor(w[:], w[:], tw_[:], op=AT.subtract)
                convacc = dp.tile([rows, 1], F32, tag="convacc", name="convacc")
                nc.vector.scalar_tensor_tensor(
                    tw_[:], w[:], 1.0, w[:], op0=AT.bypass, op1=AT.mult,
                    accum_out=convacc[:])

                a0 = dp.tile([128, 1], F32, tag="a0", name="a0")
                nc.vector.tensor_reduce(a0[:], astf_acc[:], axis=AX.X, op=AT.add)
                psa = pp.tile([1, 1], F32, tag="big", name="big")
                nc.tensor.matmul(psa[:], lhsT=a0[:], rhs=cs["ones128"],
                                 start=True, stop=True)
                psc = pp.tile([1, 1], F32, tag="big", name="big")
                nc.tensor.matmul(psc[:], lhsT=convacc[:],
                                 rhs=cs["ones64"][0:rows, :],
                                 start=True, stop=True)
                nc.scalar.copy(outt[:, 0:1], psa[:])
                nc.scalar.copy(outt[:, 1:2], psc[:])
                nc.sync.dma_start(out[:], outt[:])

    nc.finalize()
    return nc, consts


_CACHE = {}


def get_built(cdt=BF16):
    key = str(cdt)
    if key not in _CACHE:
        _CACHE[key] = build_nc(cdt=cdt)
    return _CACHE[key]


LAST_RESULT = {}


def kernel(pred_astf, true_astf, egf, target_waveform):
    import os
    import ml_dtypes
    from concourse.bass_utils import run_bass_kernel_spmd
    nc, consts = get_built(BF16)
    blobs = make_blobs(consts)
    consts = {"cblob16": blobs["cblob16"][0], "cblob32": blobs["cblob32"][0]}
    pred_astf = np.ascontiguousarray(np.asarray(pred_astf, np.float32))
    true_astf = np.ascontiguousarray(np.asarray(true_astf, np.float32))
    egf = np.ascontiguousarray(np.asarray(egf, np.float32))
    target_waveform = np.ascontiguousarray(
        np.asarray(target_waveform, np.float32))
    B = pred_astf.shape[0]
    per = B // NCORES
    in_maps = []
    for i in range(NCORES):
        sl = slice(i * per, (i + 1) * per)
        m = {"pred": pred_astf[sl], "true": true_astf[sl],
             "egf": egf[sl], "target": target_waveform[sl]}
        m.update(consts)
        in_maps.append(m)
    trace = os.environ.get("CONVALIGN_TRACE") == "1"
    res = run_bass_kernel_spmd(nc, in_maps, core_ids=list(range(NCORES)),
                               trace=trace)
    LAST_RESULT["res"] = res
    sums = np.stack([res.results[i]["out"][0] for i in range(NCORES)])
    loss_astf = np.float32(sums[:, 0].sum() / (B * L1))
    loss_conv = np.float32(sums[:, 1].sum() / (B * CROP))
    total = np.float32(loss_astf + loss_conv)
    return total, loss_astf, loss_conv
